# revision 55
# baseline (speedup 1.0000x reference)
"""Trainium2 Bass kernel for nn_CrossAttention (single-query cross attention).

Reference computation (B=4, C=64, H=W=128, heads h=64, dim_head d=64,
inner=4096, HW=16384):
    x[b, j, c]   = fimg[b, c, j]                       (j indexes H*W)
    q[b, h, d]   = sum_e fpsf[b, e] Wq[h*64+d, e]
    k[b, j, h, d]= sum_c x[b, j, c] Wk[h*64+d, c]
    out[b, h, j] = scale * sum_d q[b,h,d] k[b,j,h,d]

Single query per (batch, head) -> the attention collapses:
    W2[b, h, c]  = scale * sum_d q[b,h,d] Wk[h*64+d, c]      (tiny)
    out[b, h, j] = sum_c W2[b,h,c] fimg[b, c, j]

Sharding: j (H*W = 16384) split across 8 cores (2048 each). Every core
redundantly computes W2 (needs all heads for its output slice).

The kernel is DMA-stream bound (~1MB weights + 1MB img in, 1MB out per
core, all bf16; host casts f32<->bf16 = layout only). Trace-driven
design notes (measured on TRN2, 8 cores via axon):
  - Aggregate per-core HBM stream tops out ~350GB/s (716GB/s per stack,
    2 cores/stack running the same program). In-flight descriptors
    share that bandwidth round-robin (NOT FIFO), and each completion
    semaphore fires ~1.3-2us after the last byte (HBM receipt). So:
    few large descriptors, 128 partition rows each (64-row descriptors
    only reach ~half rate), issued in compute order -- weights first,
    so the sync queue's ~0.65us/issue serialization naturally gives
    the weights stream a head start.
  - Weights are packed into two [128, 2052] tensors (wq half on
    partitions 0:64, wk half on 64:128), each feeding one head-half of
    the A -> q2T -> B pipeline, which hides under the img stream. The
    q2T scale-copies run on scalar so the A->B handoff never queues
    behind vector's wkbd build copies.
  - Step B uses 32 block-diagonal [128x128] matmuls (2 heads each).
    The block-diag tile wkbd[d + 64par, 128p + 64par + c] is built
    on-chip: one full memset (early, off the critical path; its 3.5us
    would collide with DMA arrival if placed later) + 4 strided 3D
    copies (rearrange) from the dense wk halves (vector; scalar runs
    the big 3D builds ~2.7x slower). Assembly: bd0's four copies on
    vector (shortest path to the first big matmul), bd1's four on
    scalar so they don't interleave with vector's staging CASTs
    during the big phase (bd1 is only needed 4 matmuls in).
  - A dummy early scalar op forces the 1.3us ACT_TABLE_LOAD to run
    during the DMA wait (walrus emits it before the first ACTIVATE,
    which otherwise sits behind a late semaphore wait).
  - PSUM->SBUF staging of each big-matmul chunk is split 272/240
    across vector+scalar (balancing their rates); output leaves as
    bf16 via 4 [128, 1024] DMAs overlapping the big-matmul pipeline.
    Smaller output DMAs regress (1KB rows halve DMA efficiency).
  - Compiled with --enable-ldw-opt=true (scoped patch below): the 8
    big matmuls reuse two stationary tiles and the redundant
    per-matmul LDWEIGHTS reloads cost ~130ns each on the PE path.
  - ~25.0-25.9us NEFF exec (from 29.4us baseline); NEFF fixed
    overhead (entry barriers + program loads + exit drain) is ~15.6us
    of that, and the ~2MB-in/1MB-out stream at the ~350GB/s per-core
    HBM cap bounds most of the rest.

Device layouts (host does LAYOUT/dtype-cast only, no math):
  wA/wB [128, 2052] bf16, head-half H (heads 32H..32H+31):
      rows 0:64   = [fpsf.T | Wq.T columns for this head-half]
      rows 64:128 = [4 pad | even-head Wk blocks | odd-head Wk blocks]
        where block [d, 64p+c] = Wk[(2p+par)*64+d, c], p local pair
  img  [128, 4096] bf16: rows 64*(b%2)+c, cols 2048*(b//2)+j_local
  out  [128, 4096] bf16: rows 64*(b%2)+h, cols 2048*(b//2)+j_local

Device compute per core:
  A:  32 matmuls  q2_ps[128, 4p+b] = q2[b, 128p+r]  (lhsT = wq chunks)
  q2T [128,128] bf16 = scale * q2_ps (straight copy; rows 64*par + d
      of chunk p correspond to head 2p+par)
  wkbd[128, 4096]: wkbd[d + 64par, 128p + 64par + c] = Wk[h=2p+par][d,c]
  B:  32 matmuls  w2_ps[c + 64par, 4p+b] = W2[b, 2p+par, c]
  Assembly (8 strided vector copies): bd_q[64*half + c, 64*half + h]
      = w2_ps[c + 64par, 4p+b],  b = 2q+half, h = 2p+par
  Big: 8 matmuls [128, 512] = bd_q.T @ img chunk; psum -> bf16 SBUF
      staging (split 272/240 vector/scalar); 4 output DMAs [128, 1024].
"""

import sys
import types

import numpy as np
import ml_dtypes

# antenv.axon_hooks is absent in this image; bass_utils imports it when
# tracing. Register a minimal stand-in before importing concourse.
if "antenv.axon_hooks" not in sys.modules:
    try:
        import antenv  # noqa: F401

        _hooks = types.ModuleType("antenv.axon_hooks")
        _hooks._hook = None

        def _set_hook(h):
            _hooks._hook = h

        _hooks.set_axon_ntff_profile_hook = _set_hook
        _hooks.get_axon_ntff_profile_hook = lambda: _hooks._hook
        sys.modules["antenv.axon_hooks"] = _hooks
        try:
            from trn_agent_boot.trn_boot import _ntff_profile_via_ctypes

            _set_hook(_ntff_profile_via_ctypes("/opt/axon/libaxon_pjrt.so"))
        except Exception:
            pass
    except ImportError:
        pass

import concourse.bass as bass  # noqa: E402
import concourse.mybir as mybir  # noqa: E402
import concourse.tile as tile  # noqa: E402
from concourse import bacc  # noqa: E402
from concourse.bass_utils import run_bass_kernel_spmd  # noqa: E402

N_CORES = 8
B, C, H, W = 4, 64, 128, 128
HEADS, DIM_HEAD = 64, 64
HW = H * W
JS = HW // N_CORES  # 2048 j-positions per core
SCALE = DIM_HEAD ** -0.5
F32 = mybir.dt.float32
BF16 = mybir.dt.bfloat16
NPBF16 = ml_dtypes.bfloat16

_compiled = None  # cache (nc) across calls


def _build():
    # Enable walrus's LDWEIGHTS dedup for this kernel's compile: the 8
    # big matmuls reuse the same two [128,128] stationary tiles, and the
    # per-matmul reload costs ~130ns each on the PE's serial path.
    import concourse.bass_utils as _bu
    _orig_run = _bu.run_command

    def _run_ldwopt(cmd, **kw):
        if isinstance(cmd, list):
            cmd = ["--enable-ldw-opt=true" if c == "--enable-ldw-opt=false"
                   else c for c in cmd]
        return _orig_run(cmd, **kw)

    _bu.run_command = _run_ldwopt
    try:
        nc = _build_inner()
    finally:
        _bu.run_command = _orig_run
    return nc


def _build_inner():
    nc = bacc.Bacc("TRN2", target_bir_lowering=False, debug=False,
                   num_devices=N_CORES)

    w_d = [nc.dram_tensor(f"w{i}", [128, 2052], BF16, kind="ExternalInput")
           for i in range(2)]
    img_d = nc.dram_tensor("img", [128, 2 * JS], BF16, kind="ExternalInput")
    out_d = nc.dram_tensor("out", [128, 2 * JS], BF16, kind="ExternalOutput")

    with tile.TileContext(nc) as tc:
        with (
            tc.tile_pool(name="weights", bufs=1) as wpool,
            tc.tile_pool(name="img", bufs=1) as ipool,
            tc.tile_pool(name="small_ps", bufs=1, space="PSUM") as spsum,
            tc.tile_pool(name="big_ps", bufs=6, space="PSUM") as bpsum,
            tc.tile_pool(name="ostage", bufs=2) as opool,
        ):
            # Force the scalar ACT table load off the critical path: give
            # the ACT engine a first op whose dependency is ready almost
            # immediately, so walrus's ACT_TABLE_LOAD (1.3us) runs during
            # the DMA wait instead of right before the assembly copies.
            scr0 = wpool.tile([128, 1], F32, tag="scr0")
            scr1 = wpool.tile([128, 1], F32, tag="scr1")
            nc.vector.memset(scr0[:], 0.0)
            nc.scalar.copy(scr1[:], scr0[:])

            # Zero-fills next -- vector is idle until weights land. The
            # wkbd memset is split per column half so the ph0 build
            # copies (cols 0:2048) only wait ~1.7us of it, not all 3.5us.
            wkbd = wpool.tile([128, 4096], BF16, tag="wkbd")
            nc.vector.memset(wkbd[:, 0:2048], 0.0)
            nc.vector.memset(wkbd[:, 2048:4096], 0.0)
            bds = []
            for q in range(2):
                bd = wpool.tile([128, 128], BF16, tag=f"bd{q}")
                nc.vector.memset(bd[:], 0.0)
                bds.append(bd)

            # Input DMAs in compute order on one ring (sync HWDGE),
            # every descriptor 128 partition rows for full stream rate.
            ws = [wpool.tile([128, 2052], BF16, tag=f"w{i}", name=f"w{i}")
                  for i in range(2)]
            imgs = [ipool.tile([128, JS], BF16, tag=f"img{q}", name=f"img{q}")
                    for q in range(2)]
            for i in range(2):
                nc.sync.dma_start(ws[i][:], w_d[i].ap()[:])
            for q in range(2):
                nc.sync.dma_start(imgs[q][:], img_d.ap()[:, JS * q:JS * (q + 1)])


            q2_ps = spsum.tile([128, 128], F32, tag="q2_ps")
            w2_ps = spsum.tile([128, 128], F32, tag="w2_ps")
            q2T = wpool.tile([128, 128], BF16, tag="q2T")

            # Two half-pipelines, one per weights descriptor: A chunk ->
            # q2T copy -> B pairs, so heads 0-31 compute while the second
            # weights descriptor / img still stream in.
            for ph in range(2):
                wt = ws[ph]
                fpsfT = wt[0:64, 0:4]
                # wkbd build: per parity, fill the [64, 64] diag blocks of
                # the 16 pair-columns of this half (3D strided copy; both
                # on vector -- scalar runs this op 2.7x slower).
                for par in range(2):
                    dst = (wkbd[64 * par:64 * par + 64, :]
                           .rearrange("p (blk c) -> p blk c", c=128)
                           [:, 16 * ph:16 * ph + 16, 64 * par:64 * par + 64])
                    src = (wt[64:128, 4 + 1024 * par:4 + 1024 * par + 1024]
                           .rearrange("p (blk c) -> p blk c", c=64))
                    nc.vector.tensor_copy(dst, src)
                # A: q2_ps[r, 4p+b] = q2[b, 128p+r], local chunks 0..15.
                # The scale-folding PSUM->SBUF copy runs on scalar (so it
                # never queues behind vector's build copies) and is split
                # per 8-chunk sub-half: the first B matmuls only need the
                # first q2T quarter, so the copy of sub-half 0 overlaps
                # the A matmuls of sub-half 1 and B starts stall-free.
                for sub in range(2):
                    for lp in range(8 * sub, 8 * sub + 8):
                        p = 16 * ph + lp
                        nc.tensor.matmul(
                            q2_ps[:, 4 * p:4 * p + 4],
                            wt[0:64, 4 + 128 * lp:4 + 128 * lp + 128],
                            fpsfT,
                            start=True, stop=True,
                        )
                    nc.scalar.mul(
                        q2T[:, 64 * ph + 32 * sub:64 * ph + 32 * sub + 32],
                        q2_ps[:, 64 * ph + 32 * sub:64 * ph + 32 * sub + 32],
                        SCALE,
                    )
                # B: w2_ps[c + 64par, 4p+b] = W2[b, 2p+par, c]
                for lp in range(16):
                    p = 16 * ph + lp
                    nc.tensor.matmul(
                        w2_ps[:, 4 * p:4 * p + 4],
                        wkbd[:, 128 * p:128 * p + 128],
                        q2T[:, 4 * p:4 * p + 4],
                        start=True, stop=True,
                    )

            # Assembly: bd_q[64*half + c, 64*half + 2p+par]
            #           = w2_ps[c + 64par, 4p + 2q+half]
            # bd0 (b 0,1) on vector -- fastest path for the first big
            # matmul. bd1 (b 2,3) on scalar so those copies don't
            # interleave with vector's staging CASTs during the big phase
            # (bd1 is only needed 4 matmuls in).
            for b in [0, 1, 2, 3]:
                q, half = b // 2, b % 2
                for par in range(2):
                    dst = bds[q][64 * half:64 * half + 64,
                                 64 * half + par:64 * half + 64:2]
                    src = w2_ps[64 * par:64 * par + 64, b:128:4]
                    if q == 0:
                        nc.vector.tensor_copy(dst, src)
                    else:
                        nc.scalar.copy(dst, src)

            # Big: out rows pair q = bd_q.T @ img_q, 512-col chunks into
            # a [128, 2048] bf16 staging tile; output DMA per 1024 cols.
            for q in range(2):
                ot = opool.tile([128, JS], BF16, tag="ot")
                for k in range(4):
                    ps = bpsum.tile([128, 512], F32, tag="mm_ps")
                    nc.tensor.matmul(
                        ps[:], bds[q][:],
                        imgs[q][:, 512 * k:512 * k + 512],
                        start=True, stop=True,
                    )
                    # split each chunk's PSUM->SBUF staging across both
                    # engines (272/240: scalar runs ~11% slower per col)
                    nc.vector.tensor_copy(
                        ot[:, 512 * k:512 * k + 272], ps[:, 0:272])
                    nc.scalar.copy(
                        ot[:, 512 * k + 272:512 * k + 512], ps[:, 272:512])
                    if k % 2 == 1:
                        nc.sync.dma_start(
                            out_d.ap()[:, JS * q + 512 * (k - 1):
                                       JS * q + 512 * (k + 1)],
                            ot[:, 512 * (k - 1):512 * (k + 1)],
                        )

    nc.compile()
    return nc


def _prep_inputs(fpsf, fimg, Wq, Wk):
    fpsf = np.ascontiguousarray(fpsf, dtype=np.float32)
    fimg = np.ascontiguousarray(fimg, dtype=np.float32)
    Wq = np.ascontiguousarray(Wq, dtype=np.float32)
    Wk = np.ascontiguousarray(Wk, dtype=np.float32)

    fpsfT = fpsf.T.astype(NPBF16)
    WqT = Wq.T.astype(NPBF16)          # [64, 4096]
    Wk3 = Wk.reshape(64, 64, 64)       # [h, d, c]

    wmats = []
    for hh in range(2):
        w = np.zeros((128, 2052), NPBF16)
        w[0:64, 0:4] = fpsfT
        w[0:64, 4:2052] = WqT[:, 2048 * hh:2048 * hh + 2048]
        ev = Wk3[32 * hh:32 * hh + 32:2]       # [16, d, c] even heads
        od = Wk3[32 * hh + 1:32 * hh + 32:2]   # [16, d, c] odd heads
        w[64:128, 4:1028] = ev.transpose(1, 0, 2).reshape(64, 1024)
        w[64:128, 1028:2052] = od.transpose(1, 0, 2).reshape(64, 1024)
        wmats.append(w)

    fimg_f = fimg.reshape(B, C, HW).astype(NPBF16)
    in_maps = []
    for i in range(N_CORES):
        sh = fimg_f[:, :, JS * i:JS * (i + 1)]  # [4, 64, JS]
        # rows 64*(b%2)+c, cols JS*(b//2)+j
        img = np.ascontiguousarray(
            sh.reshape(2, 2, 64, JS).transpose(1, 2, 0, 3).reshape(128, 2 * JS)
        )
        m = {f"w{i}": wmats[i] for i in range(2)}
        m["img"] = img
        in_maps.append(m)
    return in_maps


def kernel(fpsf, fimg, Wq, Wk):
    global _compiled
    if _compiled is None:
        _compiled = _build()
    nc = _compiled

    in_maps = _prep_inputs(fpsf, fimg, Wq, Wk)
    res = run_bass_kernel_spmd(nc, in_maps, core_ids=list(range(N_CORES)))

    out = np.empty((B, HEADS, HW), dtype=np.float32)
    for i in range(N_CORES):
        r = res.results[i]["out"]  # [128, 2*JS] bf16
        out[:, :, JS * i:JS * (i + 1)] = (
            np.asarray(r).reshape(2, 64, 2, JS).transpose(2, 0, 1, 3)
            .reshape(B, HEADS, JS).astype(np.float32)
        )
    return out.reshape(B, C, H, W)


if __name__ == "__main__":
    rng = np.random.default_rng(0)
    ins = {
        "fpsf": rng.standard_normal((B, C), dtype=np.float32),
        "fimg": rng.standard_normal((B, C, H, W), dtype=np.float32),
        "Wq": (rng.standard_normal((4096, C), dtype=np.float32) * 0.05),
        "Wk": (rng.standard_normal((4096, C), dtype=np.float32) * 0.05),
    }
    out = kernel(**ins)
    print("out", out.shape, out.dtype, float(np.abs(out).max()))


# revision 56
# speedup vs baseline: 1.0142x; 1.0142x over previous
"""Trainium2 Bass kernel for nn_CrossAttention (single-query cross attention).

Reference computation (B=4, C=64, H=W=128, heads h=64, dim_head d=64,
inner=4096, HW=16384):
    x[b, j, c]   = fimg[b, c, j]                       (j indexes H*W)
    q[b, h, d]   = sum_e fpsf[b, e] Wq[h*64+d, e]
    k[b, j, h, d]= sum_c x[b, j, c] Wk[h*64+d, c]
    out[b, h, j] = scale * sum_d q[b,h,d] k[b,j,h,d]

Single query per (batch, head) -> the attention collapses:
    W2[b, h, c]  = scale * sum_d q[b,h,d] Wk[h*64+d, c]      (tiny)
    out[b, h, j] = sum_c W2[b,h,c] fimg[b, c, j]

Sharding: j (H*W = 16384) split across 8 cores (2048 each). Every core
redundantly computes W2 (needs all heads for its output slice).

The kernel is DMA-stream bound (~1MB weights + 1MB img in, 1MB out per
core, all bf16; host casts f32<->bf16 = layout only). Trace-driven
design notes (measured on TRN2, 8 cores via axon):
  - Aggregate per-core HBM stream tops out ~350GB/s (716GB/s per stack,
    2 cores/stack running the same program). In-flight descriptors
    share that bandwidth round-robin (NOT FIFO), and each completion
    semaphore fires ~1.3-2us after the last byte (HBM receipt). So:
    few large descriptors, 128 partition rows each (64-row descriptors
    only reach ~half rate), issued in compute order -- weights first,
    so the sync queue's ~0.65us/issue serialization naturally gives
    the weights stream a head start.
  - Weights are packed into two [128, 2052] tensors (wq half on
    partitions 0:64, wk half on 64:128), each feeding one head-half of
    the A -> q2T -> B pipeline, which hides under the img stream. The
    q2T scale-copies run on scalar so the A->B handoff never queues
    behind vector's wkbd build copies.
  - Step B uses 32 block-diagonal [128x128] matmuls (2 heads each).
    The block-diag tile wkbd[d + 64par, 128p + 64par + c] is built
    on-chip: one full memset (early, off the critical path; its 3.5us
    would collide with DMA arrival if placed later) + 4 strided 3D
    copies (rearrange) from the dense wk halves (vector; scalar runs
    the big 3D builds ~2.7x slower). Assembly: bd0's four copies on
    vector (shortest path to the first big matmul), bd1's four on
    scalar so they don't interleave with vector's staging CASTs
    during the big phase (bd1 is only needed 4 matmuls in).
  - A dummy early scalar op forces the 1.3us ACT_TABLE_LOAD to run
    during the DMA wait (walrus emits it before the first ACTIVATE,
    which otherwise sits behind a late semaphore wait).
  - PSUM->SBUF staging of each big-matmul chunk is split 272/240
    across vector+scalar (balancing their rates); output leaves as
    bf16 via 4 [128, 1024] DMAs overlapping the big-matmul pipeline.
    Smaller output DMAs regress (1KB rows halve DMA efficiency).
  - Compiled with --enable-ldw-opt=true (scoped patch below): the 8
    big matmuls reuse two stationary tiles and the redundant
    per-matmul LDWEIGHTS reloads cost ~130ns each on the PE path.
  - ~25.0-25.9us NEFF exec (from 29.4us baseline); NEFF fixed
    overhead (entry barriers + program loads + exit drain) is ~15.6us
    of that, and the ~2MB-in/1MB-out stream at the ~350GB/s per-core
    HBM cap bounds most of the rest.

Device layouts (host does LAYOUT/dtype-cast only, no math):
  wA/wB [128, 2052] bf16, head-half H (heads 32H..32H+31):
      rows 0:64   = [fpsf.T | Wq.T columns for this head-half]
      rows 64:128 = [4 pad | even-head Wk blocks | odd-head Wk blocks]
        where block [d, 64p+c] = Wk[(2p+par)*64+d, c], p local pair
  img  [128, 4096] bf16: rows 64*(b%2)+c, cols 2048*(b//2)+j_local
  out  [128, 4096] bf16: rows 64*(b%2)+h, cols 2048*(b//2)+j_local

Device compute per core:
  A:  32 matmuls  q2_ps[128, 4p+b] = q2[b, 128p+r]  (lhsT = wq chunks)
  q2T [128,128] bf16 = scale * q2_ps (straight copy; rows 64*par + d
      of chunk p correspond to head 2p+par)
  wkbd[128, 4096]: wkbd[d + 64par, 128p + 64par + c] = Wk[h=2p+par][d,c]
  B:  32 matmuls  w2_ps[c + 64par, 4p+b] = W2[b, 2p+par, c]
  Assembly (8 strided vector copies): bd_q[64*half + c, 64*half + h]
      = w2_ps[c + 64par, 4p+b],  b = 2q+half, h = 2p+par
  Big: 8 matmuls [128, 512] = bd_q.T @ img chunk; psum -> bf16 SBUF
      staging (split 272/240 vector/scalar); 4 output DMAs [128, 1024].
"""

import sys
import types

import numpy as np
import ml_dtypes

# antenv.axon_hooks is absent in this image; bass_utils imports it when
# tracing. Register a minimal stand-in before importing concourse.
if "antenv.axon_hooks" not in sys.modules:
    try:
        import antenv  # noqa: F401

        _hooks = types.ModuleType("antenv.axon_hooks")
        _hooks._hook = None

        def _set_hook(h):
            _hooks._hook = h

        _hooks.set_axon_ntff_profile_hook = _set_hook
        _hooks.get_axon_ntff_profile_hook = lambda: _hooks._hook
        sys.modules["antenv.axon_hooks"] = _hooks
        try:
            from trn_agent_boot.trn_boot import _ntff_profile_via_ctypes

            _set_hook(_ntff_profile_via_ctypes("/opt/axon/libaxon_pjrt.so"))
        except Exception:
            pass
    except ImportError:
        pass

import concourse.bass as bass  # noqa: E402
import concourse.mybir as mybir  # noqa: E402
import concourse.tile as tile  # noqa: E402
from concourse import bacc  # noqa: E402
from concourse.bass_utils import run_bass_kernel_spmd  # noqa: E402

N_CORES = 8
B, C, H, W = 4, 64, 128, 128
HEADS, DIM_HEAD = 64, 64
HW = H * W
JS = HW // N_CORES  # 2048 j-positions per core
SCALE = DIM_HEAD ** -0.5
F32 = mybir.dt.float32
BF16 = mybir.dt.bfloat16
NPBF16 = ml_dtypes.bfloat16

_compiled = None  # cache (nc) across calls


def _build():
    # Enable walrus's LDWEIGHTS dedup for this kernel's compile: the 8
    # big matmuls reuse the same two [128,128] stationary tiles, and the
    # per-matmul reload costs ~130ns each on the PE's serial path.
    import concourse.bass_utils as _bu
    _orig_run = _bu.run_command

    def _run_ldwopt(cmd, **kw):
        if isinstance(cmd, list):
            cmd = ["--enable-ldw-opt=true" if c == "--enable-ldw-opt=false"
                   else c for c in cmd]
        return _orig_run(cmd, **kw)

    _bu.run_command = _run_ldwopt
    try:
        nc = _build_inner()
    finally:
        _bu.run_command = _orig_run
    return nc


def _build_inner():
    nc = bacc.Bacc("TRN2", target_bir_lowering=False, debug=False,
                   num_devices=N_CORES)

    w_d = [nc.dram_tensor(f"w{i}", [128, 2052], BF16, kind="ExternalInput")
           for i in range(2)]
    img_d = nc.dram_tensor("img", [128, 2 * JS], BF16, kind="ExternalInput")
    out_d = nc.dram_tensor("out", [128, 2 * JS], BF16, kind="ExternalOutput")

    with tile.TileContext(nc) as tc:
        with (
            tc.tile_pool(name="weights", bufs=1) as wpool,
            tc.tile_pool(name="img", bufs=1) as ipool,
            tc.tile_pool(name="small_ps", bufs=1, space="PSUM") as spsum,
            tc.tile_pool(name="big_ps", bufs=6, space="PSUM") as bpsum,
            tc.tile_pool(name="ostage", bufs=2) as opool,
        ):
            # Force the scalar ACT table load off the critical path: give
            # the ACT engine a first op whose dependency is ready almost
            # immediately, so walrus's ACT_TABLE_LOAD (1.3us) runs during
            # the DMA wait instead of right before the assembly copies.
            scr0 = wpool.tile([128, 1], F32, tag="scr0")
            scr1 = wpool.tile([128, 1], F32, tag="scr1")
            nc.vector.memset(scr0[:], 0.0)
            nc.scalar.copy(scr1[:], scr0[:])

            # Zero-fills next -- vector is idle until weights land.
            wkbd = wpool.tile([128, 4096], BF16, tag="wkbd")
            nc.vector.memset(wkbd[:], 0.0)
            bds = []
            for q in range(2):
                bd = wpool.tile([128, 128], BF16, tag=f"bd{q}")
                nc.vector.memset(bd[:], 0.0)
                bds.append(bd)

            # Input DMAs in compute order on one ring (sync HWDGE),
            # every descriptor 128 partition rows for full stream rate.
            ws = [wpool.tile([128, 2052], BF16, tag=f"w{i}", name=f"w{i}")
                  for i in range(2)]
            imgs = [ipool.tile([128, JS], BF16, tag=f"img{q}", name=f"img{q}")
                    for q in range(2)]
            for i in range(2):
                nc.sync.dma_start(ws[i][:], w_d[i].ap()[:])
            for q in range(2):
                nc.sync.dma_start(imgs[q][:], img_d.ap()[:, JS * q:JS * (q + 1)])


            q2_ps = spsum.tile([128, 128], F32, tag="q2_ps")
            w2_ps = spsum.tile([128, 128], F32, tag="w2_ps")
            q2T = wpool.tile([128, 128], BF16, tag="q2T")

            # Two half-pipelines, one per weights descriptor: A chunk ->
            # q2T copy -> B pairs, so heads 0-31 compute while the second
            # weights descriptor / img still stream in.
            for ph in range(2):
                wt = ws[ph]
                fpsfT = wt[0:64, 0:4]
                # wkbd build: per parity, fill the [64, 64] diag blocks of
                # the 16 pair-columns of this half (3D strided copy; both
                # on vector -- scalar runs this op 2.7x slower).
                for par in range(2):
                    dst = (wkbd[64 * par:64 * par + 64, :]
                           .rearrange("p (blk c) -> p blk c", c=128)
                           [:, 16 * ph:16 * ph + 16, 64 * par:64 * par + 64])
                    src = (wt[64:128, 4 + 1024 * par:4 + 1024 * par + 1024]
                           .rearrange("p (blk c) -> p blk c", c=64))
                    nc.vector.tensor_copy(dst, src)
                # A: q2_ps[r, 4p+b] = q2[b, 128p+r], local chunks 0..15
                for lp in range(16):
                    p = 16 * ph + lp
                    nc.tensor.matmul(
                        q2_ps[:, 4 * p:4 * p + 4],
                        wt[0:64, 4 + 128 * lp:4 + 128 * lp + 128],
                        fpsfT,
                        start=True, stop=True,
                    )
                # scale folded into the PSUM->SBUF copy; on scalar so the
                # A->B handoff does not queue behind vector's build copies
                nc.scalar.mul(
                    q2T[:, 64 * ph:64 * ph + 64],
                    q2_ps[:, 64 * ph:64 * ph + 64],
                    SCALE,
                )
                # B: w2_ps[c + 64par, 4p+b] = W2[b, 2p+par, c]
                for lp in range(16):
                    p = 16 * ph + lp
                    nc.tensor.matmul(
                        w2_ps[:, 4 * p:4 * p + 4],
                        wkbd[:, 128 * p:128 * p + 128],
                        q2T[:, 4 * p:4 * p + 4],
                        start=True, stop=True,
                    )

            # Assembly: bd_q[64*half + c, 64*half + 2p+par]
            #           = w2_ps[c + 64par, 4p + 2q+half]
            # bd0 (b 0,1) on vector -- fastest path for the first big
            # matmul. bd1 (b 2,3) on scalar so those copies don't
            # interleave with vector's staging CASTs during the big phase
            # (bd1 is only needed 4 matmuls in).
            for b in [0, 1, 2, 3]:
                q, half = b // 2, b % 2
                for par in range(2):
                    dst = bds[q][64 * half:64 * half + 64,
                                 64 * half + par:64 * half + 64:2]
                    src = w2_ps[64 * par:64 * par + 64, b:128:4]
                    if q == 0:
                        nc.vector.tensor_copy(dst, src)
                    else:
                        nc.scalar.copy(dst, src)

            # Big: out rows pair q = bd_q.T @ img_q, 512-col chunks into
            # a [128, 2048] bf16 staging tile; output DMA per 1024 cols.
            for q in range(2):
                ot = opool.tile([128, JS], BF16, tag="ot")
                for k in range(4):
                    ps = bpsum.tile([128, 512], F32, tag="mm_ps")
                    nc.tensor.matmul(
                        ps[:], bds[q][:],
                        imgs[q][:, 512 * k:512 * k + 512],
                        start=True, stop=True,
                    )
                    # split each chunk's PSUM->SBUF staging across both
                    # engines (272/240: scalar runs ~11% slower per col)
                    nc.vector.tensor_copy(
                        ot[:, 512 * k:512 * k + 272], ps[:, 0:272])
                    nc.scalar.copy(
                        ot[:, 512 * k + 272:512 * k + 512], ps[:, 272:512])
                    if k % 2 == 1:
                        nc.sync.dma_start(
                            out_d.ap()[:, JS * q + 512 * (k - 1):
                                       JS * q + 512 * (k + 1)],
                            ot[:, 512 * (k - 1):512 * (k + 1)],
                        )

    nc.compile()
    return nc


def _prep_inputs(fpsf, fimg, Wq, Wk):
    fpsf = np.ascontiguousarray(fpsf, dtype=np.float32)
    fimg = np.ascontiguousarray(fimg, dtype=np.float32)
    Wq = np.ascontiguousarray(Wq, dtype=np.float32)
    Wk = np.ascontiguousarray(Wk, dtype=np.float32)

    fpsfT = fpsf.T.astype(NPBF16)
    WqT = Wq.T.astype(NPBF16)          # [64, 4096]
    Wk3 = Wk.reshape(64, 64, 64)       # [h, d, c]

    wmats = []
    for hh in range(2):
        w = np.zeros((128, 2052), NPBF16)
        w[0:64, 0:4] = fpsfT
        w[0:64, 4:2052] = WqT[:, 2048 * hh:2048 * hh + 2048]
        ev = Wk3[32 * hh:32 * hh + 32:2]       # [16, d, c] even heads
        od = Wk3[32 * hh + 1:32 * hh + 32:2]   # [16, d, c] odd heads
        w[64:128, 4:1028] = ev.transpose(1, 0, 2).reshape(64, 1024)
        w[64:128, 1028:2052] = od.transpose(1, 0, 2).reshape(64, 1024)
        wmats.append(w)

    fimg_f = fimg.reshape(B, C, HW).astype(NPBF16)
    in_maps = []
    for i in range(N_CORES):
        sh = fimg_f[:, :, JS * i:JS * (i + 1)]  # [4, 64, JS]
        # rows 64*(b%2)+c, cols JS*(b//2)+j
        img = np.ascontiguousarray(
            sh.reshape(2, 2, 64, JS).transpose(1, 2, 0, 3).reshape(128, 2 * JS)
        )
        m = {f"w{i}": wmats[i] for i in range(2)}
        m["img"] = img
        in_maps.append(m)
    return in_maps


def kernel(fpsf, fimg, Wq, Wk):
    global _compiled
    if _compiled is None:
        _compiled = _build()
    nc = _compiled

    in_maps = _prep_inputs(fpsf, fimg, Wq, Wk)
    res = run_bass_kernel_spmd(nc, in_maps, core_ids=list(range(N_CORES)))

    out = np.empty((B, HEADS, HW), dtype=np.float32)
    for i in range(N_CORES):
        r = res.results[i]["out"]  # [128, 2*JS] bf16
        out[:, :, JS * i:JS * (i + 1)] = (
            np.asarray(r).reshape(2, 64, 2, JS).transpose(2, 0, 1, 3)
            .reshape(B, HEADS, JS).astype(np.float32)
        )
    return out.reshape(B, C, H, W)


if __name__ == "__main__":
    rng = np.random.default_rng(0)
    ins = {
        "fpsf": rng.standard_normal((B, C), dtype=np.float32),
        "fimg": rng.standard_normal((B, C, H, W), dtype=np.float32),
        "Wq": (rng.standard_normal((4096, C), dtype=np.float32) * 0.05),
        "Wk": (rng.standard_normal((4096, C), dtype=np.float32) * 0.05),
    }
    out = kernel(**ins)
    print("out", out.shape, out.dtype, float(np.abs(out).max()))


# revision 57
# speedup vs baseline: 1.0165x; 1.0023x over previous
"""Trainium2 Bass kernel for nn_CrossAttention (single-query cross attention).

Reference computation (B=4, C=64, H=W=128, heads h=64, dim_head d=64,
inner=4096, HW=16384):
    x[b, j, c]   = fimg[b, c, j]                       (j indexes H*W)
    q[b, h, d]   = sum_e fpsf[b, e] Wq[h*64+d, e]
    k[b, j, h, d]= sum_c x[b, j, c] Wk[h*64+d, c]
    out[b, h, j] = scale * sum_d q[b,h,d] k[b,j,h,d]

Single query per (batch, head) -> the attention collapses:
    W2[b, h, c]  = scale * sum_d q[b,h,d] Wk[h*64+d, c]      (tiny)
    out[b, h, j] = sum_c W2[b,h,c] fimg[b, c, j]

Sharding: j (H*W = 16384) split across 8 cores (2048 each). Every core
redundantly computes W2 (needs all heads for its output slice).

The kernel is DMA-stream bound (~1MB weights + 1MB img in, 1MB out per
core, all bf16; host casts f32<->bf16 = layout only). Trace-driven
design notes (measured on TRN2, 8 cores via axon):
  - Aggregate per-core HBM stream tops out ~350GB/s (716GB/s per stack,
    2 cores/stack running the same program). In-flight descriptors
    share that bandwidth round-robin (NOT FIFO), and each completion
    semaphore fires ~1.3-2us after the last byte (HBM receipt). So:
    few large descriptors, 128 partition rows each (64-row descriptors
    only reach ~half rate), issued in compute order -- weights first,
    so the sync queue's ~0.65us/issue serialization naturally gives
    the weights stream a head start.
  - Weights are packed into two [128, 2052] tensors (wq half on
    partitions 0:64, wk half on 64:128), each feeding one head-half of
    the A -> q2T -> B pipeline, which hides under the img stream. The
    q2T scale-copies run on scalar so the A->B handoff never queues
    behind vector's wkbd build copies.
  - Step B uses 32 block-diagonal [128x128] matmuls (2 heads each).
    The block-diag tile wkbd[d + 64par, 128p + 64par + c] is built
    on-chip: one full memset (early, off the critical path; its 3.5us
    would collide with DMA arrival if placed later) + 4 strided 3D
    copies (rearrange) from the dense wk halves (vector; scalar runs
    the big 3D builds ~2.7x slower). Assembly: bd0's four copies on
    vector (shortest path to the first big matmul), bd1's four on
    scalar so they don't interleave with vector's staging CASTs
    during the big phase (bd1 is only needed 4 matmuls in).
  - A dummy early scalar op forces the 1.3us ACT_TABLE_LOAD to run
    during the DMA wait (walrus emits it before the first ACTIVATE,
    which otherwise sits behind a late semaphore wait).
  - PSUM->SBUF staging of each big-matmul chunk is split 272/240
    across vector+scalar (balancing their rates); output leaves as
    bf16 via 4 [128, 1024] DMAs overlapping the big-matmul pipeline.
    Smaller output DMAs regress (1KB rows halve DMA efficiency).
  - Compiled with --enable-ldw-opt=true (scoped patch below): the 8
    big matmuls reuse two stationary tiles and the redundant
    per-matmul LDWEIGHTS reloads cost ~130ns each on the PE path.
  - ~25.0-25.9us NEFF exec (from 29.4us baseline); NEFF fixed
    overhead (entry barriers + program loads + exit drain) is ~15.6us
    of that, and the ~2MB-in/1MB-out stream at the ~350GB/s per-core
    HBM cap bounds most of the rest.

Device layouts (host does LAYOUT/dtype-cast only, no math):
  wA/wB [128, 2052] bf16, head-half H (heads 32H..32H+31):
      rows 0:64   = [fpsf.T | Wq.T columns for this head-half]
      rows 64:128 = [4 pad | even-head Wk blocks | odd-head Wk blocks]
        where block [d, 64p+c] = Wk[(2p+par)*64+d, c], p local pair
  img  [128, 4096] bf16: rows 64*(b%2)+c, cols 2048*(b//2)+j_local
  out  [128, 4096] bf16: rows 64*(b%2)+h, cols 2048*(b//2)+j_local

Device compute per core:
  A:  32 matmuls  q2_ps[128, 4p+b] = q2[b, 128p+r]  (lhsT = wq chunks)
  q2T [128,128] bf16 = scale * q2_ps (straight copy; rows 64*par + d
      of chunk p correspond to head 2p+par)
  wkbd[128, 4096]: wkbd[d + 64par, 128p + 64par + c] = Wk[h=2p+par][d,c]
  B:  32 matmuls  w2_ps[c + 64par, 4p+b] = W2[b, 2p+par, c]
  Assembly (8 strided vector copies): bd_q[64*half + c, 64*half + h]
      = w2_ps[c + 64par, 4p+b],  b = 2q+half, h = 2p+par
  Big: 8 matmuls [128, 512] = bd_q.T @ img chunk; psum -> bf16 SBUF
      staging (split 272/240 vector/scalar); 4 output DMAs [128, 1024].
"""

import sys
import types

import numpy as np
import ml_dtypes

# antenv.axon_hooks is absent in this image; bass_utils imports it when
# tracing. Register a minimal stand-in before importing concourse.
if "antenv.axon_hooks" not in sys.modules:
    try:
        import antenv  # noqa: F401

        _hooks = types.ModuleType("antenv.axon_hooks")
        _hooks._hook = None

        def _set_hook(h):
            _hooks._hook = h

        _hooks.set_axon_ntff_profile_hook = _set_hook
        _hooks.get_axon_ntff_profile_hook = lambda: _hooks._hook
        sys.modules["antenv.axon_hooks"] = _hooks
        try:
            from trn_agent_boot.trn_boot import _ntff_profile_via_ctypes

            _set_hook(_ntff_profile_via_ctypes("/opt/axon/libaxon_pjrt.so"))
        except Exception:
            pass
    except ImportError:
        pass

import concourse.bass as bass  # noqa: E402
import concourse.mybir as mybir  # noqa: E402
import concourse.tile as tile  # noqa: E402
from concourse import bacc  # noqa: E402
from concourse.bass_utils import run_bass_kernel_spmd  # noqa: E402

N_CORES = 8
B, C, H, W = 4, 64, 128, 128
HEADS, DIM_HEAD = 64, 64
HW = H * W
JS = HW // N_CORES  # 2048 j-positions per core
SCALE = DIM_HEAD ** -0.5
F32 = mybir.dt.float32
BF16 = mybir.dt.bfloat16
NPBF16 = ml_dtypes.bfloat16

_compiled = None  # cache (nc) across calls


def _build():
    # Enable walrus's LDWEIGHTS dedup for this kernel's compile: the 8
    # big matmuls reuse the same two [128,128] stationary tiles, and the
    # per-matmul reload costs ~130ns each on the PE's serial path.
    import concourse.bass_utils as _bu
    _orig_run = _bu.run_command

    def _run_ldwopt(cmd, **kw):
        if isinstance(cmd, list):
            cmd = ["--enable-ldw-opt=true" if c == "--enable-ldw-opt=false"
                   else c for c in cmd]
        return _orig_run(cmd, **kw)

    _bu.run_command = _run_ldwopt
    try:
        nc = _build_inner()
    finally:
        _bu.run_command = _orig_run
    return nc


def _build_inner():
    nc = bacc.Bacc("TRN2", target_bir_lowering=False, debug=False,
                   num_devices=N_CORES)

    w_d = [nc.dram_tensor(f"w{i}", [128, 2052], BF16, kind="ExternalInput")
           for i in range(2)]
    img_d = nc.dram_tensor("img", [128, 2 * JS], BF16, kind="ExternalInput")
    out_d = nc.dram_tensor("out", [128, 2 * JS], BF16, kind="ExternalOutput")

    with tile.TileContext(nc) as tc:
        with (
            tc.tile_pool(name="weights", bufs=1) as wpool,
            tc.tile_pool(name="img", bufs=1) as ipool,
            tc.tile_pool(name="small_ps", bufs=1, space="PSUM") as spsum,
            tc.tile_pool(name="big_ps", bufs=6, space="PSUM") as bpsum,
            tc.tile_pool(name="ostage", bufs=2) as opool,
        ):
            # Force the scalar ACT table load off the critical path: give
            # the ACT engine a first op whose dependency is ready almost
            # immediately, so walrus's ACT_TABLE_LOAD (1.3us) runs during
            # the DMA wait instead of right before the assembly copies.
            scr0 = wpool.tile([128, 1], F32, tag="scr0")
            scr1 = wpool.tile([128, 1], F32, tag="scr1")
            nc.vector.memset(scr0[:], 0.0)
            nc.scalar.copy(scr1[:], scr0[:])

            # Zero-fills next -- vector is idle until weights land.
            wkbd = wpool.tile([128, 4096], BF16, tag="wkbd")
            nc.vector.memset(wkbd[:], 0.0)
            bds = []
            for q in range(2):
                bd = wpool.tile([128, 128], BF16, tag=f"bd{q}")
                nc.vector.memset(bd[:], 0.0)
                bds.append(bd)

            # Input DMAs in compute order on one ring (sync HWDGE),
            # every descriptor 128 partition rows for full stream rate.
            ws = [wpool.tile([128, 2052], BF16, tag=f"w{i}", name=f"w{i}")
                  for i in range(2)]
            imgs = [ipool.tile([128, JS], BF16, tag=f"img{q}", name=f"img{q}")
                    for q in range(2)]
            for i in range(2):
                nc.sync.dma_start(ws[i][:], w_d[i].ap()[:])
            for q in range(2):
                nc.sync.dma_start(imgs[q][:], img_d.ap()[:, JS * q:JS * (q + 1)])


            q2_ps = spsum.tile([128, 128], F32, tag="q2_ps")
            w2_ps = spsum.tile([128, 128], F32, tag="w2_ps")
            q2T = wpool.tile([128, 128], BF16, tag="q2T")

            # Two half-pipelines, one per weights descriptor: A chunk ->
            # q2T copy -> B pairs, so heads 0-31 compute while the second
            # weights descriptor / img still stream in.
            for ph in range(2):
                wt = ws[ph]
                fpsfT = wt[0:64, 0:4]
                # wkbd build: per parity, fill the [64, 64] diag blocks of
                # the 16 pair-columns of this half (3D strided copy; both
                # on vector -- scalar runs this op 2.7x slower).
                for par in range(2):
                    dst = (wkbd[64 * par:64 * par + 64, :]
                           .rearrange("p (blk c) -> p blk c", c=128)
                           [:, 16 * ph:16 * ph + 16, 64 * par:64 * par + 64])
                    src = (wt[64:128, 4 + 1024 * par:4 + 1024 * par + 1024]
                           .rearrange("p (blk c) -> p blk c", c=64))
                    nc.vector.tensor_copy(dst, src)
                # A: q2_ps[r, 4p+b] = q2[b, 128p+r], local chunks 0..15
                for lp in range(16):
                    p = 16 * ph + lp
                    nc.tensor.matmul(
                        q2_ps[:, 4 * p:4 * p + 4],
                        wt[0:64, 4 + 128 * lp:4 + 128 * lp + 128],
                        fpsfT,
                        start=True, stop=True,
                    )
                # scale folded into the PSUM->SBUF copy; on scalar so the
                # A->B handoff does not queue behind vector's build copies
                nc.scalar.mul(
                    q2T[:, 64 * ph:64 * ph + 64],
                    q2_ps[:, 64 * ph:64 * ph + 64],
                    SCALE,
                )
                # B: w2_ps[c + 64par, 4p+b] = W2[b, 2p+par, c]
                for lp in range(16):
                    p = 16 * ph + lp
                    nc.tensor.matmul(
                        w2_ps[:, 4 * p:4 * p + 4],
                        wkbd[:, 128 * p:128 * p + 128],
                        q2T[:, 4 * p:4 * p + 4],
                        start=True, stop=True,
                    )

            # Assembly: bd_q[64*half + c, 64*half + 2p+par]
            #           = w2_ps[c + 64par, 4p + 2q+half]
            # bd0 (b 0,1) on vector -- fastest path for the first big
            # matmul. bd1 (b 2,3) on scalar so those copies don't
            # interleave with vector's staging CASTs during the big phase
            # (bd1 is only needed 4 matmuls in).
            for b in [0, 1, 2, 3]:
                q, half = b // 2, b % 2
                for par in range(2):
                    dst = bds[q][64 * half:64 * half + 64,
                                 64 * half + par:64 * half + 64:2]
                    src = w2_ps[64 * par:64 * par + 64, b:128:4]
                    if q == 0:
                        nc.vector.tensor_copy(dst, src)
                    else:
                        nc.scalar.copy(dst, src)

            # Big: out rows pair q = bd_q.T @ img_q, 512-col chunks into
            # a [128, 2048] bf16 staging tile; output DMA per 1024 cols.
            for q in range(2):
                ot = opool.tile([128, JS], BF16, tag="ot")
                for k in range(4):
                    ps = bpsum.tile([128, 512], F32, tag="mm_ps")
                    nc.tensor.matmul(
                        ps[:], bds[q][:],
                        imgs[q][:, 512 * k:512 * k + 512],
                        start=True, stop=True,
                    )
                    # split each chunk's PSUM->SBUF staging across both
                    # engines (296/216: scalar is slower per column AND
                    # carries the bd1 assembly + q2T copies, so vector
                    # takes the bigger share to balance the tails)
                    nc.vector.tensor_copy(
                        ot[:, 512 * k:512 * k + 296], ps[:, 0:296])
                    nc.scalar.copy(
                        ot[:, 512 * k + 296:512 * k + 512], ps[:, 296:512])
                    if k % 2 == 1:
                        nc.sync.dma_start(
                            out_d.ap()[:, JS * q + 512 * (k - 1):
                                       JS * q + 512 * (k + 1)],
                            ot[:, 512 * (k - 1):512 * (k + 1)],
                        )

    nc.compile()
    return nc


def _prep_inputs(fpsf, fimg, Wq, Wk):
    fpsf = np.ascontiguousarray(fpsf, dtype=np.float32)
    fimg = np.ascontiguousarray(fimg, dtype=np.float32)
    Wq = np.ascontiguousarray(Wq, dtype=np.float32)
    Wk = np.ascontiguousarray(Wk, dtype=np.float32)

    fpsfT = fpsf.T.astype(NPBF16)
    WqT = Wq.T.astype(NPBF16)          # [64, 4096]
    Wk3 = Wk.reshape(64, 64, 64)       # [h, d, c]

    wmats = []
    for hh in range(2):
        w = np.zeros((128, 2052), NPBF16)
        w[0:64, 0:4] = fpsfT
        w[0:64, 4:2052] = WqT[:, 2048 * hh:2048 * hh + 2048]
        ev = Wk3[32 * hh:32 * hh + 32:2]       # [16, d, c] even heads
        od = Wk3[32 * hh + 1:32 * hh + 32:2]   # [16, d, c] odd heads
        w[64:128, 4:1028] = ev.transpose(1, 0, 2).reshape(64, 1024)
        w[64:128, 1028:2052] = od.transpose(1, 0, 2).reshape(64, 1024)
        wmats.append(w)

    fimg_f = fimg.reshape(B, C, HW).astype(NPBF16)
    in_maps = []
    for i in range(N_CORES):
        sh = fimg_f[:, :, JS * i:JS * (i + 1)]  # [4, 64, JS]
        # rows 64*(b%2)+c, cols JS*(b//2)+j
        img = np.ascontiguousarray(
            sh.reshape(2, 2, 64, JS).transpose(1, 2, 0, 3).reshape(128, 2 * JS)
        )
        m = {f"w{i}": wmats[i] for i in range(2)}
        m["img"] = img
        in_maps.append(m)
    return in_maps


def kernel(fpsf, fimg, Wq, Wk):
    global _compiled
    if _compiled is None:
        _compiled = _build()
    nc = _compiled

    in_maps = _prep_inputs(fpsf, fimg, Wq, Wk)
    res = run_bass_kernel_spmd(nc, in_maps, core_ids=list(range(N_CORES)))

    out = np.empty((B, HEADS, HW), dtype=np.float32)
    for i in range(N_CORES):
        r = res.results[i]["out"]  # [128, 2*JS] bf16
        out[:, :, JS * i:JS * (i + 1)] = (
            np.asarray(r).reshape(2, 64, 2, JS).transpose(2, 0, 1, 3)
            .reshape(B, HEADS, JS).astype(np.float32)
        )
    return out.reshape(B, C, H, W)


if __name__ == "__main__":
    rng = np.random.default_rng(0)
    ins = {
        "fpsf": rng.standard_normal((B, C), dtype=np.float32),
        "fimg": rng.standard_normal((B, C, H, W), dtype=np.float32),
        "Wq": (rng.standard_normal((4096, C), dtype=np.float32) * 0.05),
        "Wk": (rng.standard_normal((4096, C), dtype=np.float32) * 0.05),
    }
    out = kernel(**ins)
    print("out", out.shape, out.dtype, float(np.abs(out).max()))


# revision 58
# speedup vs baseline: 1.0340x; 1.0172x over previous
"""Trainium2 Bass kernel for nn_CrossAttention (single-query cross attention).

Reference computation (B=4, C=64, H=W=128, heads h=64, dim_head d=64,
inner=4096, HW=16384):
    x[b, j, c]   = fimg[b, c, j]                       (j indexes H*W)
    q[b, h, d]   = sum_e fpsf[b, e] Wq[h*64+d, e]
    k[b, j, h, d]= sum_c x[b, j, c] Wk[h*64+d, c]
    out[b, h, j] = scale * sum_d q[b,h,d] k[b,j,h,d]

Single query per (batch, head) -> the attention collapses:
    W2[b, h, c]  = scale * sum_d q[b,h,d] Wk[h*64+d, c]      (tiny)
    out[b, h, j] = sum_c W2[b,h,c] fimg[b, c, j]

Sharding: j (H*W = 16384) split across 8 cores (2048 each). Every core
redundantly computes W2 (needs all heads for its output slice).

The kernel is DMA-stream bound (~1MB weights + 1MB img in, 1MB out per
core, all bf16; host casts f32<->bf16 = layout only). Trace-driven
design notes (measured on TRN2, 8 cores via axon):
  - Aggregate per-core HBM stream tops out ~350GB/s (716GB/s per stack,
    2 cores/stack running the same program). In-flight descriptors
    share that bandwidth round-robin (NOT FIFO), and each completion
    semaphore fires ~1.3-2us after the last byte (HBM receipt). So:
    few large descriptors, 128 partition rows each (64-row descriptors
    only reach ~half rate), issued in compute order -- weights first,
    so the sync queue's ~0.65us/issue serialization naturally gives
    the weights stream a head start.
  - Weights are packed into two [128, 2052] tensors (wq half on
    partitions 0:64, wk half on 64:128), each feeding one head-half of
    the A -> q2T -> B pipeline, which hides under the img stream. The
    q2T scale-copies run on scalar so the A->B handoff never queues
    behind vector's wkbd build copies.
  - Step B uses 32 block-diagonal [128x128] matmuls (2 heads each).
    The block-diag tile wkbd[d + 64par, 128p + 64par + c] is built
    on-chip: one full memset (early, off the critical path; its 3.5us
    would collide with DMA arrival if placed later) + 4 strided 3D
    copies (rearrange) from the dense wk halves (vector; scalar runs
    the big 3D builds ~2.7x slower). Assembly: bd0's four copies on
    vector (shortest path to the first big matmul), bd1's four on
    scalar so they don't interleave with vector's staging CASTs
    during the big phase (bd1 is only needed 4 matmuls in).
  - A dummy early scalar op forces the 1.3us ACT_TABLE_LOAD to run
    during the DMA wait (walrus emits it before the first ACTIVATE,
    which otherwise sits behind a late semaphore wait).
  - PSUM->SBUF staging of each big-matmul chunk is split 272/240
    across vector+scalar (balancing their rates); output leaves as
    bf16 via 4 [128, 1024] DMAs overlapping the big-matmul pipeline.
    Smaller output DMAs regress (1KB rows halve DMA efficiency).
  - Compiled with --enable-ldw-opt=true (scoped patch below): the 8
    big matmuls reuse two stationary tiles and the redundant
    per-matmul LDWEIGHTS reloads cost ~130ns each on the PE path.
  - ~25.0-25.9us NEFF exec (from 29.4us baseline); NEFF fixed
    overhead (entry barriers + program loads + exit drain) is ~15.6us
    of that, and the ~2MB-in/1MB-out stream at the ~350GB/s per-core
    HBM cap bounds most of the rest.

Device layouts (host does LAYOUT/dtype-cast only, no math):
  wA/wB [128, 2052] bf16, head-half H (heads 32H..32H+31):
      rows 0:64   = [fpsf.T | Wq.T columns for this head-half]
      rows 64:128 = [4 pad | even-head Wk blocks | odd-head Wk blocks]
        where block [d, 64p+c] = Wk[(2p+par)*64+d, c], p local pair
  img  [128, 4096] bf16: rows 64*(b%2)+c, cols 2048*(b//2)+j_local
  out  [128, 4096] bf16: rows 64*(b%2)+h, cols 2048*(b//2)+j_local

Device compute per core:
  A:  32 matmuls  q2_ps[128, 4p+b] = q2[b, 128p+r]  (lhsT = wq chunks)
  q2T [128,128] bf16 = scale * q2_ps (straight copy; rows 64*par + d
      of chunk p correspond to head 2p+par)
  wkbd[128, 4096]: wkbd[d + 64par, 128p + 64par + c] = Wk[h=2p+par][d,c]
  B:  32 matmuls  w2_ps[c + 64par, 4p+b] = W2[b, 2p+par, c]
  Assembly (8 strided vector copies): bd_q[64*half + c, 64*half + h]
      = w2_ps[c + 64par, 4p+b],  b = 2q+half, h = 2p+par
  Big: 8 matmuls [128, 512] = bd_q.T @ img chunk; psum -> bf16 SBUF
      staging (split 272/240 vector/scalar); 4 output DMAs [128, 1024].
"""

import sys
import types

import numpy as np
import ml_dtypes

# antenv.axon_hooks is absent in this image; bass_utils imports it when
# tracing. Register a minimal stand-in before importing concourse.
if "antenv.axon_hooks" not in sys.modules:
    try:
        import antenv  # noqa: F401

        _hooks = types.ModuleType("antenv.axon_hooks")
        _hooks._hook = None

        def _set_hook(h):
            _hooks._hook = h

        _hooks.set_axon_ntff_profile_hook = _set_hook
        _hooks.get_axon_ntff_profile_hook = lambda: _hooks._hook
        sys.modules["antenv.axon_hooks"] = _hooks
        try:
            from trn_agent_boot.trn_boot import _ntff_profile_via_ctypes

            _set_hook(_ntff_profile_via_ctypes("/opt/axon/libaxon_pjrt.so"))
        except Exception:
            pass
    except ImportError:
        pass

import concourse.bass as bass  # noqa: E402
import concourse.mybir as mybir  # noqa: E402
import concourse.tile as tile  # noqa: E402
from concourse import bacc  # noqa: E402
from concourse.bass_utils import run_bass_kernel_spmd  # noqa: E402

N_CORES = 8
B, C, H, W = 4, 64, 128, 128
HEADS, DIM_HEAD = 64, 64
HW = H * W
JS = HW // N_CORES  # 2048 j-positions per core
SCALE = DIM_HEAD ** -0.5
F32 = mybir.dt.float32
BF16 = mybir.dt.bfloat16
NPBF16 = ml_dtypes.bfloat16

_compiled = None  # cache (nc) across calls


def _build():
    # Enable walrus's LDWEIGHTS dedup for this kernel's compile: the 8
    # big matmuls reuse the same two [128,128] stationary tiles, and the
    # per-matmul reload costs ~130ns each on the PE's serial path.
    import concourse.bass_utils as _bu
    _orig_run = _bu.run_command

    def _run_ldwopt(cmd, **kw):
        if isinstance(cmd, list):
            cmd = ["--enable-ldw-opt=true" if c == "--enable-ldw-opt=false"
                   else c for c in cmd]
        return _orig_run(cmd, **kw)

    _bu.run_command = _run_ldwopt
    try:
        nc = _build_inner()
    finally:
        _bu.run_command = _orig_run
    return nc


def _build_inner():
    nc = bacc.Bacc("TRN2", target_bir_lowering=False, debug=False,
                   num_devices=N_CORES)

    w_d = [nc.dram_tensor(f"w{i}", [128, 2052], BF16, kind="ExternalInput")
           for i in range(2)]
    img_d = nc.dram_tensor("img", [128, 2 * JS], BF16, kind="ExternalInput")
    out_d = nc.dram_tensor("out", [128, 2 * JS], BF16, kind="ExternalOutput")

    with tile.TileContext(nc) as tc:
        with (
            tc.tile_pool(name="weights", bufs=1) as wpool,
            tc.tile_pool(name="img", bufs=1) as ipool,
            tc.tile_pool(name="small_ps", bufs=1, space="PSUM") as spsum,
            tc.tile_pool(name="big_ps", bufs=6, space="PSUM") as bpsum,
            tc.tile_pool(name="ostage", bufs=2) as opool,
        ):
            # Force the scalar ACT table load off the critical path: give
            # the ACT engine a first op whose dependency is ready almost
            # immediately, so walrus's ACT_TABLE_LOAD (1.3us) runs during
            # the DMA wait instead of right before the assembly copies.
            scr0 = wpool.tile([128, 1], F32, tag="scr0")
            scr1 = wpool.tile([128, 1], F32, tag="scr1")
            nc.vector.memset(scr0[:], 0.0)
            nc.scalar.copy(scr1[:], scr0[:])

            # Zero-fills next -- vector is idle until weights land.
            wkbd = wpool.tile([128, 4096], BF16, tag="wkbd")
            nc.vector.memset(wkbd[:], 0.0)
            bds = []
            for q in range(2):
                bd = wpool.tile([128, 128], BF16, tag=f"bd{q}")
                nc.vector.memset(bd[:], 0.0)
                bds.append(bd)

            # Input DMAs in compute order on one ring (sync HWDGE),
            # every descriptor 128 partition rows for full stream rate.
            ws = [wpool.tile([128, 2052], BF16, tag=f"w{i}", name=f"w{i}")
                  for i in range(2)]
            imgs = [ipool.tile([128, JS], BF16, tag=f"img{q}", name=f"img{q}")
                    for q in range(2)]
            for i in range(2):
                nc.sync.dma_start(ws[i][:], w_d[i].ap()[:])
            for q in range(2):
                nc.sync.dma_start(imgs[q][:], img_d.ap()[:, JS * q:JS * (q + 1)])


            q2_ps = spsum.tile([128, 128], F32, tag="q2_ps")
            w2_ps = spsum.tile([128, 128], F32, tag="w2_ps")
            q2T = wpool.tile([128, 128], BF16, tag="q2T")

            # Two half-pipelines, one per weights descriptor: A chunk ->
            # q2T copy -> B pairs, so heads 0-31 compute while the second
            # weights descriptor / img still stream in.
            for ph in range(2):
                wt = ws[ph]
                fpsfT = wt[0:64, 0:4]
                # wkbd build: per parity, fill the [64, 64] diag blocks of
                # the 16 pair-columns of this half (3D strided copy; both
                # on vector -- scalar runs this op 2.7x slower).
                for par in range(2):
                    dst = (wkbd[64 * par:64 * par + 64, :]
                           .rearrange("p (blk c) -> p blk c", c=128)
                           [:, 16 * ph:16 * ph + 16, 64 * par:64 * par + 64])
                    src = (wt[64:128, 4 + 1024 * par:4 + 1024 * par + 1024]
                           .rearrange("p (blk c) -> p blk c", c=64))
                    nc.vector.tensor_copy(dst, src)
                # A: q2_ps[r, 4p+b] = q2[b, 128p+r], local chunks 0..15
                for lp in range(16):
                    p = 16 * ph + lp
                    nc.tensor.matmul(
                        q2_ps[:, 4 * p:4 * p + 4],
                        wt[0:64, 4 + 128 * lp:4 + 128 * lp + 128],
                        fpsfT,
                        start=True, stop=True,
                    )
                # scale folded into the PSUM->SBUF copy; on scalar so the
                # A->B handoff does not queue behind vector's build copies
                nc.scalar.mul(
                    q2T[:, 64 * ph:64 * ph + 64],
                    q2_ps[:, 64 * ph:64 * ph + 64],
                    SCALE,
                )
                # B: w2_ps[c + 64par, 4p+b] = W2[b, 2p+par, c]
                for lp in range(16):
                    p = 16 * ph + lp
                    nc.tensor.matmul(
                        w2_ps[:, 4 * p:4 * p + 4],
                        wkbd[:, 128 * p:128 * p + 128],
                        q2T[:, 4 * p:4 * p + 4],
                        start=True, stop=True,
                    )

            # Assembly: bd_q[64*half + c, 64*half + 2p+par]
            #           = w2_ps[c + 64par, 4p + 2q+half]
            # bd0 (b 0,1) on vector -- fastest path for the first big
            # matmul. bd1 (b 2,3) on scalar so those copies don't
            # interleave with vector's staging CASTs during the big phase
            # (bd1 is only needed 4 matmuls in).
            for b in [0, 1, 2, 3]:
                q, half = b // 2, b % 2
                for par in range(2):
                    dst = bds[q][64 * half:64 * half + 64,
                                 64 * half + par:64 * half + 64:2]
                    src = w2_ps[64 * par:64 * par + 64, b:128:4]
                    if q == 0:
                        nc.vector.tensor_copy(dst, src)
                    else:
                        nc.scalar.copy(dst, src)

            # Big: out rows pair q = bd_q.T @ img_q, 512-col chunks into
            # a [128, 2048] bf16 staging tile; output DMA per 1024 cols.
            for q in range(2):
                ot = opool.tile([128, JS], BF16, tag="ot")
                for k in range(4):
                    ps = bpsum.tile([128, 512], F32, tag="mm_ps")
                    nc.tensor.matmul(
                        ps[:], bds[q][:],
                        imgs[q][:, 512 * k:512 * k + 512],
                        start=True, stop=True,
                    )
                    # split each chunk's PSUM->SBUF staging across both
                    # engines (272/240: scalar runs ~11% slower per col)
                    nc.vector.tensor_copy(
                        ot[:, 512 * k:512 * k + 272], ps[:, 0:272])
                    nc.scalar.copy(
                        ot[:, 512 * k + 272:512 * k + 512], ps[:, 272:512])
                    if k % 2 == 1:
                        nc.sync.dma_start(
                            out_d.ap()[:, JS * q + 512 * (k - 1):
                                       JS * q + 512 * (k + 1)],
                            ot[:, 512 * (k - 1):512 * (k + 1)],
                        )

    nc.compile()
    return nc


def _prep_inputs(fpsf, fimg, Wq, Wk):
    fpsf = np.ascontiguousarray(fpsf, dtype=np.float32)
    fimg = np.ascontiguousarray(fimg, dtype=np.float32)
    Wq = np.ascontiguousarray(Wq, dtype=np.float32)
    Wk = np.ascontiguousarray(Wk, dtype=np.float32)

    fpsfT = fpsf.T.astype(NPBF16)
    WqT = Wq.T.astype(NPBF16)          # [64, 4096]
    Wk3 = Wk.reshape(64, 64, 64)       # [h, d, c]

    wmats = []
    for hh in range(2):
        w = np.zeros((128, 2052), NPBF16)
        w[0:64, 0:4] = fpsfT
        w[0:64, 4:2052] = WqT[:, 2048 * hh:2048 * hh + 2048]
        ev = Wk3[32 * hh:32 * hh + 32:2]       # [16, d, c] even heads
        od = Wk3[32 * hh + 1:32 * hh + 32:2]   # [16, d, c] odd heads
        w[64:128, 4:1028] = ev.transpose(1, 0, 2).reshape(64, 1024)
        w[64:128, 1028:2052] = od.transpose(1, 0, 2).reshape(64, 1024)
        wmats.append(w)

    fimg_f = fimg.reshape(B, C, HW).astype(NPBF16)
    in_maps = []
    for i in range(N_CORES):
        sh = fimg_f[:, :, JS * i:JS * (i + 1)]  # [4, 64, JS]
        # rows 64*(b%2)+c, cols JS*(b//2)+j
        img = np.ascontiguousarray(
            sh.reshape(2, 2, 64, JS).transpose(1, 2, 0, 3).reshape(128, 2 * JS)
        )
        m = {f"w{i}": wmats[i] for i in range(2)}
        m["img"] = img
        in_maps.append(m)
    return in_maps


def kernel(fpsf, fimg, Wq, Wk):
    global _compiled
    if _compiled is None:
        _compiled = _build()
    nc = _compiled

    in_maps = _prep_inputs(fpsf, fimg, Wq, Wk)
    res = run_bass_kernel_spmd(nc, in_maps, core_ids=list(range(N_CORES)))

    out = np.empty((B, HEADS, HW), dtype=np.float32)
    for i in range(N_CORES):
        r = res.results[i]["out"]  # [128, 2*JS] bf16
        out[:, :, JS * i:JS * (i + 1)] = (
            np.asarray(r).reshape(2, 64, 2, JS).transpose(2, 0, 1, 3)
            .reshape(B, HEADS, JS).astype(np.float32)
        )
    return out.reshape(B, C, H, W)


if __name__ == "__main__":
    rng = np.random.default_rng(0)
    ins = {
        "fpsf": rng.standard_normal((B, C), dtype=np.float32),
        "fimg": rng.standard_normal((B, C, H, W), dtype=np.float32),
        "Wq": (rng.standard_normal((4096, C), dtype=np.float32) * 0.05),
        "Wk": (rng.standard_normal((4096, C), dtype=np.float32) * 0.05),
    }
    out = kernel(**ins)
    print("out", out.shape, out.dtype, float(np.abs(out).max()))


# revision 59
# speedup vs baseline: 1.0458x; 1.0114x over previous
"""Trainium2 Bass kernel for nn_CrossAttention (single-query cross attention).

Reference computation (B=4, C=64, H=W=128, heads h=64, dim_head d=64,
inner=4096, HW=16384):
    x[b, j, c]   = fimg[b, c, j]                       (j indexes H*W)
    q[b, h, d]   = sum_e fpsf[b, e] Wq[h*64+d, e]
    k[b, j, h, d]= sum_c x[b, j, c] Wk[h*64+d, c]
    out[b, h, j] = scale * sum_d q[b,h,d] k[b,j,h,d]

Single query per (batch, head) -> the attention collapses:
    W2[b, h, c]  = scale * sum_d q[b,h,d] Wk[h*64+d, c]      (tiny)
    out[b, h, j] = sum_c W2[b,h,c] fimg[b, c, j]

Sharding: j (H*W = 16384) split across 8 cores (2048 each). Every core
redundantly computes W2 (needs all heads for its output slice).

The kernel is DMA-stream bound (~1MB weights + 1MB img in, 1MB out per
core, all bf16; host casts f32<->bf16 = layout only). Trace-driven
design notes (measured on TRN2, 8 cores via axon):
  - Aggregate per-core HBM stream tops out ~350GB/s (716GB/s per stack,
    2 cores/stack running the same program). In-flight descriptors
    share that bandwidth round-robin (NOT FIFO), and each completion
    semaphore fires ~1.3-2us after the last byte (HBM receipt). So:
    few large descriptors, 128 partition rows each (64-row descriptors
    only reach ~half rate), issued in compute order -- weights first,
    so the sync queue's ~0.65us/issue serialization naturally gives
    the weights stream a head start.
  - Weights are packed into two [128, 2052] tensors (wq half on
    partitions 0:64, wk half on 64:128), each feeding one head-half of
    the A -> q2T -> B pipeline, which hides under the img stream. The
    q2T scale-copies run on scalar so the A->B handoff never queues
    behind vector's wkbd build copies.
  - Step B uses 32 block-diagonal [128x128] matmuls (2 heads each).
    The block-diag tile wkbd[d + 64par, 128p + 64par + c] is built
    on-chip: one full memset (early, off the critical path; its 3.5us
    would collide with DMA arrival if placed later) + 4 strided 3D
    copies (rearrange) from the dense wk halves (vector; scalar runs
    the big 3D builds ~2.7x slower). Assembly: bd0's four copies on
    vector (shortest path to the first big matmul), bd1's four on
    scalar so they don't interleave with vector's staging CASTs
    during the big phase (bd1 is only needed 4 matmuls in).
  - A dummy early scalar op forces the 1.3us ACT_TABLE_LOAD to run
    during the DMA wait (walrus emits it before the first ACTIVATE,
    which otherwise sits behind a late semaphore wait).
  - PSUM->SBUF staging of each big-matmul chunk is split 272/240
    across vector+scalar (balancing their rates); output leaves as
    bf16 via 4 [128, 1024] DMAs overlapping the big-matmul pipeline.
    Smaller output DMAs regress (1KB rows halve DMA efficiency).
  - Compiled with --enable-ldw-opt=true (scoped patch below): the 8
    big matmuls reuse two stationary tiles and the redundant
    per-matmul LDWEIGHTS reloads cost ~130ns each on the PE path.
  - ~25.0-25.9us NEFF exec (from 29.4us baseline); NEFF fixed
    overhead (entry barriers + program loads + exit drain) is ~15.6us
    of that, and the ~2MB-in/1MB-out stream at the ~350GB/s per-core
    HBM cap bounds most of the rest.

Device layouts (host does LAYOUT/dtype-cast only, no math):
  wA/wB [128, 2052] bf16, head-half H (heads 32H..32H+31):
      rows 0:64   = [fpsf.T | Wq.T columns for this head-half]
      rows 64:128 = [4 pad | even-head Wk blocks | odd-head Wk blocks]
        where block [d, 64p+c] = Wk[(2p+par)*64+d, c], p local pair
  img  [128, 4096] bf16: rows 64*(b%2)+c, cols 2048*(b//2)+j_local
  out  [128, 4096] bf16: rows 64*(b%2)+h, cols 2048*(b//2)+j_local

Device compute per core:
  A:  32 matmuls  q2_ps[128, 4p+b] = q2[b, 128p+r]  (lhsT = wq chunks)
  q2T [128,128] bf16 = scale * q2_ps (straight copy; rows 64*par + d
      of chunk p correspond to head 2p+par)
  wkbd[128, 4096]: wkbd[d + 64par, 128p + 64par + c] = Wk[h=2p+par][d,c]
  B:  32 matmuls  w2_ps[c + 64par, 4p+b] = W2[b, 2p+par, c]
  Assembly (8 strided vector copies): bd_q[64*half + c, 64*half + h]
      = w2_ps[c + 64par, 4p+b],  b = 2q+half, h = 2p+par
  Big: 8 matmuls [128, 512] = bd_q.T @ img chunk; psum -> bf16 SBUF
      staging (split 272/240 vector/scalar); 4 output DMAs [128, 1024].
"""

import sys
import types

import numpy as np
import ml_dtypes

# antenv.axon_hooks is absent in this image; bass_utils imports it when
# tracing. Register a minimal stand-in before importing concourse.
if "antenv.axon_hooks" not in sys.modules:
    try:
        import antenv  # noqa: F401

        _hooks = types.ModuleType("antenv.axon_hooks")
        _hooks._hook = None

        def _set_hook(h):
            _hooks._hook = h

        _hooks.set_axon_ntff_profile_hook = _set_hook
        _hooks.get_axon_ntff_profile_hook = lambda: _hooks._hook
        sys.modules["antenv.axon_hooks"] = _hooks
        try:
            from trn_agent_boot.trn_boot import _ntff_profile_via_ctypes

            _set_hook(_ntff_profile_via_ctypes("/opt/axon/libaxon_pjrt.so"))
        except Exception:
            pass
    except ImportError:
        pass

import concourse.bass as bass  # noqa: E402
import concourse.mybir as mybir  # noqa: E402
import concourse.tile as tile  # noqa: E402
from concourse import bacc  # noqa: E402
from concourse.bass_utils import run_bass_kernel_spmd  # noqa: E402

N_CORES = 8
B, C, H, W = 4, 64, 128, 128
HEADS, DIM_HEAD = 64, 64
HW = H * W
JS = HW // N_CORES  # 2048 j-positions per core
SCALE = DIM_HEAD ** -0.5
F32 = mybir.dt.float32
BF16 = mybir.dt.bfloat16
NPBF16 = ml_dtypes.bfloat16

_compiled = None  # cache (nc) across calls


def _build():
    # Enable walrus's LDWEIGHTS dedup for this kernel's compile: the 8
    # big matmuls reuse the same two [128,128] stationary tiles, and the
    # per-matmul reload costs ~130ns each on the PE's serial path.
    import concourse.bass_utils as _bu
    _orig_run = _bu.run_command

    def _run_ldwopt(cmd, **kw):
        if isinstance(cmd, list):
            cmd = ["--enable-ldw-opt=true" if c == "--enable-ldw-opt=false"
                   else c for c in cmd]
        return _orig_run(cmd, **kw)

    _bu.run_command = _run_ldwopt
    try:
        nc = _build_inner()
    finally:
        _bu.run_command = _orig_run
    return nc


def _build_inner():
    nc = bacc.Bacc("TRN2", target_bir_lowering=False, debug=False,
                   num_devices=N_CORES)

    w_d = [nc.dram_tensor(f"w{i}", [128, 2052], BF16, kind="ExternalInput")
           for i in range(2)]
    img_d = nc.dram_tensor("img", [128, 2 * JS], BF16, kind="ExternalInput")
    out_d = nc.dram_tensor("out", [128, 2 * JS], BF16, kind="ExternalOutput")

    with tile.TileContext(nc) as tc:
        with (
            tc.tile_pool(name="weights", bufs=1) as wpool,
            tc.tile_pool(name="img", bufs=1) as ipool,
            tc.tile_pool(name="small_ps", bufs=1, space="PSUM") as spsum,
            tc.tile_pool(name="big_ps", bufs=6, space="PSUM") as bpsum,
            tc.tile_pool(name="ostage", bufs=2) as opool,
        ):
            # Force the scalar ACT table load off the critical path: give
            # the ACT engine a first op whose dependency is ready almost
            # immediately, so walrus's ACT_TABLE_LOAD (1.3us) runs during
            # the DMA wait instead of right before the assembly copies.
            scr0 = wpool.tile([128, 1], F32, tag="scr0")
            scr1 = wpool.tile([128, 1], F32, tag="scr1")
            nc.vector.memset(scr0[:], 0.0)
            nc.scalar.copy(scr1[:], scr0[:])

            # Zero-fills next -- vector is idle until weights land.
            wkbd = wpool.tile([128, 4096], BF16, tag="wkbd")
            nc.vector.memset(wkbd[:], 0.0)
            bds = []
            for q in range(2):
                bd = wpool.tile([128, 128], BF16, tag=f"bd{q}")
                nc.vector.memset(bd[:], 0.0)
                bds.append(bd)

            # Input DMAs in compute order on one ring (sync HWDGE),
            # every descriptor 128 partition rows for full stream rate.
            ws = [wpool.tile([128, 2052], BF16, tag=f"w{i}", name=f"w{i}")
                  for i in range(2)]
            img_s = ipool.tile([128, 2 * JS], BF16, tag="img_s")
            imgs = [img_s[:, JS * q:JS * (q + 1)] for q in range(2)]
            for i in range(2):
                nc.sync.dma_start(ws[i][:], w_d[i].ap()[:])
            # one img descriptor: with two, both halves finish near the
            # stream end anyway (round-robin sharing) but pair 1's
            # separate completion semaphore fires ~1.4us later still and
            # stalled big-matmul k4; one descriptor = one earlier sem.
            nc.sync.dma_start(img_s[:], img_d.ap()[:])


            q2_ps = spsum.tile([128, 128], F32, tag="q2_ps")
            w2_ps = spsum.tile([128, 128], F32, tag="w2_ps")
            q2T = wpool.tile([128, 128], BF16, tag="q2T")

            # Two half-pipelines, one per weights descriptor: A chunk ->
            # q2T copy -> B pairs, so heads 0-31 compute while the second
            # weights descriptor / img still stream in.
            for ph in range(2):
                wt = ws[ph]
                fpsfT = wt[0:64, 0:4]
                # wkbd build: per parity, fill the [64, 64] diag blocks of
                # the 16 pair-columns of this half (3D strided copy; both
                # on vector -- scalar runs this op 2.7x slower).
                for par in range(2):
                    dst = (wkbd[64 * par:64 * par + 64, :]
                           .rearrange("p (blk c) -> p blk c", c=128)
                           [:, 16 * ph:16 * ph + 16, 64 * par:64 * par + 64])
                    src = (wt[64:128, 4 + 1024 * par:4 + 1024 * par + 1024]
                           .rearrange("p (blk c) -> p blk c", c=64))
                    nc.vector.tensor_copy(dst, src)
                # A: q2_ps[r, 4p+b] = q2[b, 128p+r], local chunks 0..15
                for lp in range(16):
                    p = 16 * ph + lp
                    nc.tensor.matmul(
                        q2_ps[:, 4 * p:4 * p + 4],
                        wt[0:64, 4 + 128 * lp:4 + 128 * lp + 128],
                        fpsfT,
                        start=True, stop=True,
                    )
                # scale folded into the PSUM->SBUF copy; on scalar so the
                # A->B handoff does not queue behind vector's build copies
                nc.scalar.mul(
                    q2T[:, 64 * ph:64 * ph + 64],
                    q2_ps[:, 64 * ph:64 * ph + 64],
                    SCALE,
                )
                # B: w2_ps[c + 64par, 4p+b] = W2[b, 2p+par, c]
                for lp in range(16):
                    p = 16 * ph + lp
                    nc.tensor.matmul(
                        w2_ps[:, 4 * p:4 * p + 4],
                        wkbd[:, 128 * p:128 * p + 128],
                        q2T[:, 4 * p:4 * p + 4],
                        start=True, stop=True,
                    )

            # Assembly: bd_q[64*half + c, 64*half + 2p+par]
            #           = w2_ps[c + 64par, 4p + 2q+half]
            # bd0 (b 0,1) on vector -- fastest path for the first big
            # matmul. bd1 (b 2,3) on scalar so those copies don't
            # interleave with vector's staging CASTs during the big phase
            # (bd1 is only needed 4 matmuls in).
            for b in [0, 1, 2, 3]:
                q, half = b // 2, b % 2
                for par in range(2):
                    dst = bds[q][64 * half:64 * half + 64,
                                 64 * half + par:64 * half + 64:2]
                    src = w2_ps[64 * par:64 * par + 64, b:128:4]
                    if q == 0:
                        nc.vector.tensor_copy(dst, src)
                    else:
                        nc.scalar.copy(dst, src)

            # Big: out rows pair q = bd_q.T @ img_q, 512-col chunks into
            # a [128, 2048] bf16 staging tile; output DMA per 1024 cols.
            for q in range(2):
                ot = opool.tile([128, JS], BF16, tag="ot")
                for k in range(4):
                    ps = bpsum.tile([128, 512], F32, tag="mm_ps")
                    nc.tensor.matmul(
                        ps[:], bds[q][:],
                        imgs[q][:, 512 * k:512 * k + 512],
                        start=True, stop=True,
                    )
                    # split each chunk's PSUM->SBUF staging across both
                    # engines (272/240: scalar runs ~11% slower per col)
                    nc.vector.tensor_copy(
                        ot[:, 512 * k:512 * k + 272], ps[:, 0:272])
                    nc.scalar.copy(
                        ot[:, 512 * k + 272:512 * k + 512], ps[:, 272:512])
                    if k % 2 == 1:
                        nc.sync.dma_start(
                            out_d.ap()[:, JS * q + 512 * (k - 1):
                                       JS * q + 512 * (k + 1)],
                            ot[:, 512 * (k - 1):512 * (k + 1)],
                        )

    nc.compile()
    return nc


def _prep_inputs(fpsf, fimg, Wq, Wk):
    fpsf = np.ascontiguousarray(fpsf, dtype=np.float32)
    fimg = np.ascontiguousarray(fimg, dtype=np.float32)
    Wq = np.ascontiguousarray(Wq, dtype=np.float32)
    Wk = np.ascontiguousarray(Wk, dtype=np.float32)

    fpsfT = fpsf.T.astype(NPBF16)
    WqT = Wq.T.astype(NPBF16)          # [64, 4096]
    Wk3 = Wk.reshape(64, 64, 64)       # [h, d, c]

    wmats = []
    for hh in range(2):
        w = np.zeros((128, 2052), NPBF16)
        w[0:64, 0:4] = fpsfT
        w[0:64, 4:2052] = WqT[:, 2048 * hh:2048 * hh + 2048]
        ev = Wk3[32 * hh:32 * hh + 32:2]       # [16, d, c] even heads
        od = Wk3[32 * hh + 1:32 * hh + 32:2]   # [16, d, c] odd heads
        w[64:128, 4:1028] = ev.transpose(1, 0, 2).reshape(64, 1024)
        w[64:128, 1028:2052] = od.transpose(1, 0, 2).reshape(64, 1024)
        wmats.append(w)

    fimg_f = fimg.reshape(B, C, HW).astype(NPBF16)
    in_maps = []
    for i in range(N_CORES):
        sh = fimg_f[:, :, JS * i:JS * (i + 1)]  # [4, 64, JS]
        # rows 64*(b%2)+c, cols JS*(b//2)+j
        img = np.ascontiguousarray(
            sh.reshape(2, 2, 64, JS).transpose(1, 2, 0, 3).reshape(128, 2 * JS)
        )
        m = {f"w{i}": wmats[i] for i in range(2)}
        m["img"] = img
        in_maps.append(m)
    return in_maps


def kernel(fpsf, fimg, Wq, Wk):
    global _compiled
    if _compiled is None:
        _compiled = _build()
    nc = _compiled

    in_maps = _prep_inputs(fpsf, fimg, Wq, Wk)
    res = run_bass_kernel_spmd(nc, in_maps, core_ids=list(range(N_CORES)))

    out = np.empty((B, HEADS, HW), dtype=np.float32)
    for i in range(N_CORES):
        r = res.results[i]["out"]  # [128, 2*JS] bf16
        out[:, :, JS * i:JS * (i + 1)] = (
            np.asarray(r).reshape(2, 64, 2, JS).transpose(2, 0, 1, 3)
            .reshape(B, HEADS, JS).astype(np.float32)
        )
    return out.reshape(B, C, H, W)


if __name__ == "__main__":
    rng = np.random.default_rng(0)
    ins = {
        "fpsf": rng.standard_normal((B, C), dtype=np.float32),
        "fimg": rng.standard_normal((B, C, H, W), dtype=np.float32),
        "Wq": (rng.standard_normal((4096, C), dtype=np.float32) * 0.05),
        "Wk": (rng.standard_normal((4096, C), dtype=np.float32) * 0.05),
    }
    out = kernel(**ins)
    print("out", out.shape, out.dtype, float(np.abs(out).max()))


# revision 60
# speedup vs baseline: 1.0487x; 1.0028x over previous
"""Trainium2 Bass kernel for nn_CrossAttention (single-query cross attention).

Reference computation (B=4, C=64, H=W=128, heads h=64, dim_head d=64,
inner=4096, HW=16384):
    x[b, j, c]   = fimg[b, c, j]                       (j indexes H*W)
    q[b, h, d]   = sum_e fpsf[b, e] Wq[h*64+d, e]
    k[b, j, h, d]= sum_c x[b, j, c] Wk[h*64+d, c]
    out[b, h, j] = scale * sum_d q[b,h,d] k[b,j,h,d]

Single query per (batch, head) -> the attention collapses:
    W2[b, h, c]  = scale * sum_d q[b,h,d] Wk[h*64+d, c]      (tiny)
    out[b, h, j] = sum_c W2[b,h,c] fimg[b, c, j]

Sharding: j (H*W = 16384) split across 8 cores (2048 each). Every core
redundantly computes W2 (needs all heads for its output slice).

The kernel is DMA-stream bound (~1MB weights + 1MB img in, 1MB out per
core, all bf16; host casts f32<->bf16 = layout only). Trace-driven
design notes (measured on TRN2, 8 cores via axon):
  - Aggregate per-core HBM stream tops out ~350GB/s (716GB/s per stack,
    2 cores/stack running the same program). In-flight descriptors
    share that bandwidth round-robin (NOT FIFO), and each completion
    semaphore fires ~1.3-2us after the last byte (HBM receipt). So:
    few large descriptors, 128 partition rows each (64-row descriptors
    only reach ~half rate), issued in compute order -- weights first,
    so the sync queue's ~0.65us/issue serialization naturally gives
    the weights stream a head start.
  - Weights are packed into two [128, 2052] tensors (wq half on
    partitions 0:64, wk half on 64:128), each feeding one head-half of
    the A -> q2T -> B pipeline, which hides under the img stream. The
    q2T scale-copies run on scalar so the A->B handoff never queues
    behind vector's wkbd build copies. img ships as ONE [128, 4096]
    descriptor: with two, both halves finished near the stream end
    anyway (round-robin) but pair 1's separate completion semaphore
    fired ~1.4us later still and stalled big-matmul k4.
  - Step B uses 32 block-diagonal [128x128] matmuls (2 heads each).
    The block-diag tile wkbd[d + 64par, 128p + 64par + c] is built
    on-chip: one full memset (early, off the critical path; its 3.5us
    would collide with DMA arrival if placed later) + 4 strided 3D
    copies (rearrange) from the dense wk halves (vector; scalar runs
    the big 3D builds ~2.7x slower). Assembly: bd0's four copies on
    vector (shortest path to the first big matmul), bd1's four on
    scalar so they don't interleave with vector's staging CASTs
    during the big phase (bd1 is only needed 4 matmuls in).
  - A dummy early scalar op forces the 1.3us ACT_TABLE_LOAD to run
    during the DMA wait (walrus emits it before the first ACTIVATE,
    which otherwise sits behind a late semaphore wait).
  - PSUM->SBUF staging of each big-matmul chunk is split 272/240
    across vector+scalar (balancing their rates); output leaves as
    bf16 via 4 [128, 1024] DMAs overlapping the big-matmul pipeline.
    Smaller output DMAs regress (1KB rows halve DMA efficiency).
  - Compiled with --enable-ldw-opt=true (scoped patch below): the 8
    big matmuls reuse two stationary tiles and the redundant
    per-matmul LDWEIGHTS reloads cost ~130ns each on the PE path.
  - ~25.0-25.9us NEFF exec (from 29.4us baseline); NEFF fixed
    overhead (entry barriers + program loads + exit drain) is ~15.6us
    of that, and the ~2MB-in/1MB-out stream at the ~350GB/s per-core
    HBM cap bounds most of the rest.

Device layouts (host does LAYOUT/dtype-cast only, no math):
  wA/wB [128, 2052] bf16, head-half H (heads 32H..32H+31):
      rows 0:64   = [fpsf.T | Wq.T columns for this head-half]
      rows 64:128 = [4 pad | even-head Wk blocks | odd-head Wk blocks]
        where block [d, 64p+c] = Wk[(2p+par)*64+d, c], p local pair
  img  [128, 4096] bf16: rows 64*(b%2)+c, cols 2048*(b//2)+j_local
  out  [128, 4096] bf16: rows 64*(b%2)+h, cols 2048*(b//2)+j_local

Device compute per core:
  A:  32 matmuls  q2_ps[128, 4p+b] = q2[b, 128p+r]  (lhsT = wq chunks)
  q2T [128,128] bf16 = scale * q2_ps (straight copy; rows 64*par + d
      of chunk p correspond to head 2p+par)
  wkbd[128, 4096]: wkbd[d + 64par, 128p + 64par + c] = Wk[h=2p+par][d,c]
  B:  32 matmuls  w2_ps[c + 64par, 4p+b] = W2[b, 2p+par, c]
  Assembly (8 strided vector copies): bd_q[64*half + c, 64*half + h]
      = w2_ps[c + 64par, 4p+b],  b = 2q+half, h = 2p+par
  Big: 8 matmuls [128, 512] = bd_q.T @ img chunk; psum -> bf16 SBUF
      staging (split 272/240 vector/scalar); 4 output DMAs [128, 1024].
"""

import sys
import types

import numpy as np
import ml_dtypes

# antenv.axon_hooks is absent in this image; bass_utils imports it when
# tracing. Register a minimal stand-in before importing concourse.
if "antenv.axon_hooks" not in sys.modules:
    try:
        import antenv  # noqa: F401

        _hooks = types.ModuleType("antenv.axon_hooks")
        _hooks._hook = None

        def _set_hook(h):
            _hooks._hook = h

        _hooks.set_axon_ntff_profile_hook = _set_hook
        _hooks.get_axon_ntff_profile_hook = lambda: _hooks._hook
        sys.modules["antenv.axon_hooks"] = _hooks
        try:
            from trn_agent_boot.trn_boot import _ntff_profile_via_ctypes

            _set_hook(_ntff_profile_via_ctypes("/opt/axon/libaxon_pjrt.so"))
        except Exception:
            pass
    except ImportError:
        pass

import concourse.bass as bass  # noqa: E402
import concourse.mybir as mybir  # noqa: E402
import concourse.tile as tile  # noqa: E402
from concourse import bacc  # noqa: E402
from concourse.bass_utils import run_bass_kernel_spmd  # noqa: E402

N_CORES = 8
B, C, H, W = 4, 64, 128, 128
HEADS, DIM_HEAD = 64, 64
HW = H * W
JS = HW // N_CORES  # 2048 j-positions per core
SCALE = DIM_HEAD ** -0.5
F32 = mybir.dt.float32
BF16 = mybir.dt.bfloat16
NPBF16 = ml_dtypes.bfloat16

_compiled = None  # cache (nc) across calls


def _build():
    # Enable walrus's LDWEIGHTS dedup for this kernel's compile: the 8
    # big matmuls reuse the same two [128,128] stationary tiles, and the
    # per-matmul reload costs ~130ns each on the PE's serial path.
    import concourse.bass_utils as _bu
    _orig_run = _bu.run_command

    def _run_ldwopt(cmd, **kw):
        if isinstance(cmd, list):
            cmd = ["--enable-ldw-opt=true" if c == "--enable-ldw-opt=false"
                   else c for c in cmd]
        return _orig_run(cmd, **kw)

    _bu.run_command = _run_ldwopt
    try:
        nc = _build_inner()
    finally:
        _bu.run_command = _orig_run
    return nc


def _build_inner():
    nc = bacc.Bacc("TRN2", target_bir_lowering=False, debug=False,
                   num_devices=N_CORES)

    w_d = [nc.dram_tensor(f"w{i}", [128, 2052], BF16, kind="ExternalInput")
           for i in range(2)]
    img_d = nc.dram_tensor("img", [128, 2 * JS], BF16, kind="ExternalInput")
    out_d = nc.dram_tensor("out", [128, 2 * JS], BF16, kind="ExternalOutput")

    with tile.TileContext(nc) as tc:
        with (
            tc.tile_pool(name="weights", bufs=1) as wpool,
            tc.tile_pool(name="img", bufs=1) as ipool,
            tc.tile_pool(name="small_ps", bufs=1, space="PSUM") as spsum,
            tc.tile_pool(name="big_ps", bufs=6, space="PSUM") as bpsum,
            tc.tile_pool(name="ostage", bufs=2) as opool,
        ):
            # Force the scalar ACT table load off the critical path: give
            # the ACT engine a first op whose dependency is ready almost
            # immediately, so walrus's ACT_TABLE_LOAD (1.3us) runs during
            # the DMA wait instead of right before the assembly copies.
            scr0 = wpool.tile([128, 1], F32, tag="scr0")
            scr1 = wpool.tile([128, 1], F32, tag="scr1")
            nc.vector.memset(scr0[:], 0.0)
            nc.scalar.copy(scr1[:], scr0[:])

            # Zero-fills next -- vector is idle until weights land.
            wkbd = wpool.tile([128, 4096], BF16, tag="wkbd")
            nc.vector.memset(wkbd[:], 0.0)
            bds = []
            for q in range(2):
                bd = wpool.tile([128, 128], BF16, tag=f"bd{q}")
                nc.vector.memset(bd[:], 0.0)
                bds.append(bd)

            # Input DMAs in compute order on one ring (sync HWDGE),
            # every descriptor 128 partition rows for full stream rate.
            ws = [wpool.tile([128, 2052], BF16, tag=f"w{i}", name=f"w{i}")
                  for i in range(2)]
            img_s = ipool.tile([128, 2 * JS], BF16, tag="img_s")
            imgs = [img_s[:, JS * q:JS * (q + 1)] for q in range(2)]
            for i in range(2):
                nc.sync.dma_start(ws[i][:], w_d[i].ap()[:])
            # one img descriptor: with two, both halves finish near the
            # stream end anyway (round-robin sharing) but pair 1's
            # separate completion semaphore fires ~1.4us later still and
            # stalled big-matmul k4; one descriptor = one earlier sem.
            nc.sync.dma_start(img_s[:], img_d.ap()[:])


            q2_ps = spsum.tile([128, 128], F32, tag="q2_ps")
            w2_ps = spsum.tile([128, 128], F32, tag="w2_ps")
            q2T = wpool.tile([128, 128], BF16, tag="q2T")

            # Two half-pipelines, one per weights descriptor: A chunk ->
            # q2T copy -> B pairs, so heads 0-31 compute while the second
            # weights descriptor / img still stream in.
            for ph in range(2):
                wt = ws[ph]
                fpsfT = wt[0:64, 0:4]
                # wkbd build: per parity, fill the [64, 64] diag blocks of
                # the 16 pair-columns of this half (3D strided copy; both
                # on vector -- scalar runs this op 2.7x slower).
                for par in range(2):
                    dst = (wkbd[64 * par:64 * par + 64, :]
                           .rearrange("p (blk c) -> p blk c", c=128)
                           [:, 16 * ph:16 * ph + 16, 64 * par:64 * par + 64])
                    src = (wt[64:128, 4 + 1024 * par:4 + 1024 * par + 1024]
                           .rearrange("p (blk c) -> p blk c", c=64))
                    nc.vector.tensor_copy(dst, src)
                # A: q2_ps[r, 4p+b] = q2[b, 128p+r], local chunks 0..15
                for lp in range(16):
                    p = 16 * ph + lp
                    nc.tensor.matmul(
                        q2_ps[:, 4 * p:4 * p + 4],
                        wt[0:64, 4 + 128 * lp:4 + 128 * lp + 128],
                        fpsfT,
                        start=True, stop=True,
                    )
                # scale folded into the PSUM->SBUF copy; on scalar so the
                # A->B handoff does not queue behind vector's build copies
                nc.scalar.mul(
                    q2T[:, 64 * ph:64 * ph + 64],
                    q2_ps[:, 64 * ph:64 * ph + 64],
                    SCALE,
                )
                # B: w2_ps[c + 64par, 4p+b] = W2[b, 2p+par, c]
                for lp in range(16):
                    p = 16 * ph + lp
                    nc.tensor.matmul(
                        w2_ps[:, 4 * p:4 * p + 4],
                        wkbd[:, 128 * p:128 * p + 128],
                        q2T[:, 4 * p:4 * p + 4],
                        start=True, stop=True,
                    )

            # Assembly: bd_q[64*half + c, 64*half + 2p+par]
            #           = w2_ps[c + 64par, 4p + 2q+half]
            # bd0 (b 0,1) on vector -- fastest path for the first big
            # matmul. bd1 (b 2,3) on scalar so those copies don't
            # interleave with vector's staging CASTs during the big phase
            # (bd1 is only needed 4 matmuls in).
            for b in [0, 1, 2, 3]:
                q, half = b // 2, b % 2
                for par in range(2):
                    dst = bds[q][64 * half:64 * half + 64,
                                 64 * half + par:64 * half + 64:2]
                    src = w2_ps[64 * par:64 * par + 64, b:128:4]
                    if q == 0:
                        nc.vector.tensor_copy(dst, src)
                    else:
                        nc.scalar.copy(dst, src)

            # Big: out rows pair q = bd_q.T @ img_q, 512-col chunks into
            # a [128, 2048] bf16 staging tile; output DMA per 1024 cols.
            for q in range(2):
                ot = opool.tile([128, JS], BF16, tag="ot")
                for k in range(4):
                    ps = bpsum.tile([128, 512], F32, tag="mm_ps")
                    nc.tensor.matmul(
                        ps[:], bds[q][:],
                        imgs[q][:, 512 * k:512 * k + 512],
                        start=True, stop=True,
                    )
                    # split each chunk's PSUM->SBUF staging across both
                    # engines (272/240: scalar runs ~11% slower per col)
                    nc.vector.tensor_copy(
                        ot[:, 512 * k:512 * k + 272], ps[:, 0:272])
                    nc.scalar.copy(
                        ot[:, 512 * k + 272:512 * k + 512], ps[:, 272:512])
                    if k % 2 == 1:
                        nc.sync.dma_start(
                            out_d.ap()[:, JS * q + 512 * (k - 1):
                                       JS * q + 512 * (k + 1)],
                            ot[:, 512 * (k - 1):512 * (k + 1)],
                        )

    nc.compile()
    return nc


def _prep_inputs(fpsf, fimg, Wq, Wk):
    fpsf = np.ascontiguousarray(fpsf, dtype=np.float32)
    fimg = np.ascontiguousarray(fimg, dtype=np.float32)
    Wq = np.ascontiguousarray(Wq, dtype=np.float32)
    Wk = np.ascontiguousarray(Wk, dtype=np.float32)

    fpsfT = fpsf.T.astype(NPBF16)
    WqT = Wq.T.astype(NPBF16)          # [64, 4096]
    Wk3 = Wk.reshape(64, 64, 64)       # [h, d, c]

    wmats = []
    for hh in range(2):
        w = np.zeros((128, 2052), NPBF16)
        w[0:64, 0:4] = fpsfT
        w[0:64, 4:2052] = WqT[:, 2048 * hh:2048 * hh + 2048]
        ev = Wk3[32 * hh:32 * hh + 32:2]       # [16, d, c] even heads
        od = Wk3[32 * hh + 1:32 * hh + 32:2]   # [16, d, c] odd heads
        w[64:128, 4:1028] = ev.transpose(1, 0, 2).reshape(64, 1024)
        w[64:128, 1028:2052] = od.transpose(1, 0, 2).reshape(64, 1024)
        wmats.append(w)

    fimg_f = fimg.reshape(B, C, HW).astype(NPBF16)
    in_maps = []
    for i in range(N_CORES):
        sh = fimg_f[:, :, JS * i:JS * (i + 1)]  # [4, 64, JS]
        # rows 64*(b%2)+c, cols JS*(b//2)+j
        img = np.ascontiguousarray(
            sh.reshape(2, 2, 64, JS).transpose(1, 2, 0, 3).reshape(128, 2 * JS)
        )
        m = {f"w{i}": wmats[i] for i in range(2)}
        m["img"] = img
        in_maps.append(m)
    return in_maps


def kernel(fpsf, fimg, Wq, Wk):
    global _compiled
    if _compiled is None:
        _compiled = _build()
    nc = _compiled

    in_maps = _prep_inputs(fpsf, fimg, Wq, Wk)
    res = run_bass_kernel_spmd(nc, in_maps, core_ids=list(range(N_CORES)))

    out = np.empty((B, HEADS, HW), dtype=np.float32)
    for i in range(N_CORES):
        r = res.results[i]["out"]  # [128, 2*JS] bf16
        out[:, :, JS * i:JS * (i + 1)] = (
            np.asarray(r).reshape(2, 64, 2, JS).transpose(2, 0, 1, 3)
            .reshape(B, HEADS, JS).astype(np.float32)
        )
    return out.reshape(B, C, H, W)


if __name__ == "__main__":
    rng = np.random.default_rng(0)
    ins = {
        "fpsf": rng.standard_normal((B, C), dtype=np.float32),
        "fimg": rng.standard_normal((B, C, H, W), dtype=np.float32),
        "Wq": (rng.standard_normal((4096, C), dtype=np.float32) * 0.05),
        "Wk": (rng.standard_normal((4096, C), dtype=np.float32) * 0.05),
    }
    out = kernel(**ins)
    print("out", out.shape, out.dtype, float(np.abs(out).max()))
